# revision 44
# baseline (speedup 1.0000x reference)
"""GAT layer (PyG GATConv-style) on 8 Trainium2 NeuronCores.

Strategy:
- Nodes sharded across 8 cores by destination; edges partitioned by destination
  node in a partition-aligned layout: each destination node owns one SBUF
  partition of its block; its incoming edges sit along the free dim, padded to
  a per-block uniform length L (degree-sorted bin packing keeps padding ~1.3%).
- Host expands x[src] per edge slot in fp8-e4m3 (sharding-time data movement)
  with a k-half-interleaved layout [pf][kh][l][p] that doubles as the DoubleRow
  matmul operand layout: per-edge h and a_src each take ONE DoubleRow fp8
  matmul (k=256 in a single pass, 0.5 cyc/row). Folded weights are fp8-e4m3
  scaled x4 to dodge subnormals; the 1/4 is undone exactly via the exp scale
  (attention path) and the 1/s normalization (h path).
- Every node's self-loop sits at edge column 0, so the edge stream itself
  carries each node's own features: a_dst pre-fills the block's a_src PSUM
  bank by a matmul with lhsT = the l=0 slice and a broadcast-tiled Wtd rhs
  (no separate own-feature input, no logits add), and one extra DoubleRow
  matmul drops per-node a_dst into spare agg-bank columns for the padding
  correction. ACT computes e = exp(leaky(z)) straight from PSUM.
- Padding slots are exact zeros; their softmax contribution is subtracted
  analytically (host pad-count x device exp(leaky(a_dst))).

Scheduling (what makes it fast): cross-engine syncs are per-engine COUNTING
semaphores and every engine queue is in-order, so ANY instruction that waits
freezes its engine's counter for every later consumer. The whole kernel is
therefore software-pipelined so that no emitted instruction can wait on
same-period work:
- the block front-end (a_src bank + exp(leaky(z))) runs THREE blocks ahead
  of the h-projection groups, so alpha weights are always old when the
  DVE multiplies dispatch;
- h-projections fill 16-column PSUM superblocks (two banks, one accumulation
  group each) consumed by ONE DVE multiply per superblock;
- the weighted-h and softmax-sum aggregation matmuls (DoubleRow fp8 identity
  pairs / bf16 identity columns into a shared 6-block [h|e|a_dst] PSUM bank)
  are deferred THREE blocks so they are ready wherever the list scheduler
  places them; a_dst lands in its own block and is the last edge-tile reader;
- the finalize (pad fix, 1/(4s) scale, log-softmax row sums) is itself
  pipelined one cross-engine stage per block, reading the group bank in
  place (GPSIMD never touches PSUM - the BIR verifier forbids it);
- all log-softmax flushes batch at the end: Ln lives in a different ACT
  table set than Prelu/Exp, and in-loop flushes would thrash 1.3us table
  reloads twice per flush.

kernel(**inputs) takes FULL inputs and returns the FULL [N, 64] output.
"""

import numpy as np
import ml_dtypes

import concourse.bass as bass
import concourse.bacc as bacc
import concourse.tile as tile
from concourse import mybir
from concourse.bass_utils import run_bass_kernel_spmd
from concourse.masks import make_identity

# Problem shape (hardcoded per contract)
N, F, E = 100000, 256, 1600000
H, C = 8, 8
HC = H * C  # 64
NEG_SLOPE = 0.2
SC_H = 4.0   # fp8 weight scale for the h projection
SC_A = 4.0   # fp8/bf16 weight scale for the attention projections

P = 128
NCORES = 8
NB = 98                      # blocks per core
NPC = NB * P                 # 12544 node slots per core
NSLOT = NCORES * NPC         # 100352 >= N
TDA = HC + 2 * H             # 80: agg layout [h(64) | e(8) | a_dst(8)]

bf16 = ml_dtypes.bfloat16
f8 = ml_dtypes.float8_e4m3


def _host_prep(x, edge_index, W, att_src, att_dst, bias):
    src_e = np.asarray(edge_index[0], dtype=np.int64)
    dst_e = np.asarray(edge_index[1], dtype=np.int64)
    loop = np.arange(N, dtype=np.int64)
    # self-loops FIRST so each node's own features sit at edge column l=0
    src = np.concatenate([loop, src_e])
    dst = np.concatenate([loop, dst_e])

    deg = np.bincount(dst, minlength=N).astype(np.int64)

    # nodes sorted by degree desc -> global 128-slot blocks dealt round-robin
    # to cores so every core's j-th block has (nearly) equal max degree.
    order = np.argsort(-deg, kind="stable")
    ks = np.arange(NSLOT)
    g = ks // P
    p = ks % P
    c = g % NCORES
    j = g // NCORES
    rows = c * NPC + j * P + p          # device row of global sorted slot k
    row2node = np.full(NSLOT, -1, dtype=np.int64)
    row2node[rows[:N]] = order
    node2row = np.empty(N, dtype=np.int64)
    node2row[order] = rows[:N]

    # per-core-block uniform L schedule (exact max over the 8-block group)
    deg_slot = np.zeros(NSLOT, dtype=np.int64)
    deg_slot[:N] = deg[order]           # degree of global sorted slot k
    degb = deg_slot.reshape(NSLOT // P, P).max(axis=1)   # per global block g
    L_sched = degb.reshape(NB, NCORES).max(axis=1)
    L_sched = np.maximum(L_sched, 1)
    assert L_sched.max() <= 63, "a_src PSUM bank holds at most 63 edge columns"
    off = np.zeros(NB + 1, dtype=np.int64)
    off[1:] = np.cumsum(P * L_sched)
    S = int(off[-1])                    # edge slots per core

    # pad count per device row (for the analytic softmax-denominator fix)
    deg_row = np.zeros(NSLOT, dtype=np.int64)
    deg_row[rows] = deg_slot

    # folded weights
    Wt = np.asarray(W, dtype=np.float64).T            # [256, 64]
    att_s = np.asarray(att_src, np.float64)           # [8, 8]
    att_d = np.asarray(att_dst, np.float64)
    Wts = np.stack([Wt[:, h * C:(h + 1) * C] @ att_s[h] for h in range(H)], axis=1)  # [256, 8]
    Wtd = np.stack([Wt[:, h * C:(h + 1) * C] @ att_d[h] for h in range(H)], axis=1)  # [256, 8]

    def il(a, dtype):  # feature-interleave rows: [256, d] -> [128, 2, d]
        d = a.shape[1]
        return np.ascontiguousarray(
            a.reshape(2, P, d).transpose(1, 0, 2).astype(np.float32).astype(dtype))

    wf8 = il(Wt * SC_H, f8)            # [128, 2, 64] fp8, x4
    was8 = il(Wts * SC_A, f8)          # [128, 2, 8]  fp8, x4
    wtd8 = il(Wtd * SC_A, f8)          # [128, 2, 8]  fp8, x4 (per-node a_dst)
    wdb = il(Wtd * SC_A, bf16)         # [128, 2, 8]  bf16, x4 (a_dst prefill)
    assert np.abs(Wt * SC_H).max() < 400 and np.abs(Wts * SC_A).max() < 400

    # DoubleRow identity: idr[p, i, m] = (m == p) for both k-tiles i
    idr = np.zeros((P, 2, P), dtype=f8)
    idr[np.arange(P), :, np.arange(P)] = 1.0

    # edge -> slot (vectorized); slot storage order (j, l, p)
    eorder = np.argsort(dst, kind="stable")
    dst_s = dst[eorder]
    src_s = src[eorder]
    starts = np.zeros(N + 1, dtype=np.int64)
    starts[1:] = np.cumsum(deg)
    l_rank = np.arange(len(dst_s), dtype=np.int64) - starts[dst_s]
    r = node2row[dst_s]
    ec = r // NPC
    within = r % NPC
    ej = within // P
    ep = within % P
    pos = off[ej] + l_rank * P + ep

    x_f8 = np.asarray(x, np.float32).astype(f8)
    assert np.abs(np.asarray(x, np.float32)).max() < 400  # e4m3 range

    bias_np = np.asarray(bias, np.float32)
    has_bias = bool(np.any(bias_np != 0.0))
    bias_rep = np.tile(bias_np.reshape(1, HC), (P, 1))

    in_maps = []
    for cc in range(NCORES):
        m = ec == cc
        xe = np.zeros((S, F), dtype=f8)               # pad slots stay zero
        xe[pos[m]] = x_f8[src_s[m]]
        # per block: [L, P, F] -> [F, L*P] -> k-half interleave [128, 2, L*P]
        parts = []
        for jj in range(NB):
            Lj = int(L_sched[jj])
            a = xe[off[jj]:off[jj + 1]].reshape(Lj, P, F)      # [l, p, f]
            a = a.transpose(2, 0, 1).reshape(2, P, Lj * P)     # [kh*128f, l*p]
            a = a.transpose(1, 0, 2)                           # [128f, kh, l*p]
            parts.append(np.ascontiguousarray(a).reshape(-1))
        xeT = np.concatenate(parts)
        del xe

        # pad slots per row: [P, NB]
        d = deg_row[cc * NPC:(cc + 1) * NPC].reshape(NB, P)
        npad = (L_sched[:, None] - d).T.astype(np.float32).astype(bf16)

        in_maps.append({
            "xeT": xeT,
            "wf8": wf8,
            "was8": was8,
            "wtd8": wtd8,
            "wdb": wdb,
            "idr": np.ascontiguousarray(idr.reshape(P, 2 * P)),
            "bias_rep": bias_rep,
            "npad": np.ascontiguousarray(npad),
        })
    return in_maps, L_sched, S, row2node, has_bias


def _build_program(L_sched, S, BX=12, BH=2, B2=7, BW=12, KPRE=6,
                   POOL_EVERY=4, has_bias=False):
    nc = bacc.Bacc("TRN2", target_bir_lowering=False, debug=False,
                   enable_asserts=False, num_devices=NCORES)
    dt = mybir.dt
    DR = mybir.MatmulPerfMode.DoubleRow

    xeT = nc.dram_tensor("xeT", [S * 2 * P], dt.float8e4, kind="ExternalInput").ap()
    wf8 = nc.dram_tensor("wf8", [P, 2, HC], dt.float8e4, kind="ExternalInput").ap()
    was8 = nc.dram_tensor("was8", [P, 2, H], dt.float8e4, kind="ExternalInput").ap()
    wtd8 = nc.dram_tensor("wtd8", [P, 2, H], dt.float8e4, kind="ExternalInput").ap()
    wdb = nc.dram_tensor("wdb", [P, 2, H], dt.bfloat16, kind="ExternalInput").ap()
    idr = nc.dram_tensor("idr", [P, 2 * P], dt.float8e4, kind="ExternalInput").ap()
    bias_rep = nc.dram_tensor("bias_rep", [P, HC], dt.float32, kind="ExternalInput").ap()
    npad = nc.dram_tensor("npad", [P, NB], dt.bfloat16, kind="ExternalInput").ap()
    out = nc.dram_tensor("out", [P, NB * HC], dt.float16, kind="ExternalOutput").ap()

    AF = mybir.ActivationFunctionType
    OP = mybir.AluOpType
    GP8 = 8   # l-group: 8 x 64 fp32 fills one 2KB PSUM bank
    GB = 6    # finalize group: blocks normalized together (6*80 fp32 = one bank)
    FB = 14   # output flush chunk (blocks)
    ISC = 1.0 / SC_A   # exp input scale undoing the x4 attention-weight scale

    with tile.TileContext(nc) as tc:
        with (
            tc.tile_pool(name="const", bufs=1) as constp,
            tc.tile_pool(name="resid", bufs=1) as residp,
            tc.tile_pool(name="p2xpre", bufs=KPRE) as p2xpre,
        ):
            wf8_t = constp.tile([P, 2, HC], dt.float8e4)
            nc.scalar.dma_start(wf8_t[:], wf8[:])
            was8_t = constp.tile([P, 2, H], dt.float8e4)
            nc.scalar.dma_start(was8_t[:], was8[:])
            wtd8_t = constp.tile([P, 2, H], dt.float8e4)
            nc.scalar.dma_start(wtd8_t[:], wtd8[:])
            wdb_t = constp.tile([P, 2, H], dt.bfloat16)
            nc.scalar.dma_start(wdb_t[:], wdb[:])
            idr_t = constp.tile([P, 2, P], dt.float8e4)
            nc.scalar.dma_start(idr_t[:], idr[:].rearrange("p (k q) -> p k q", k=2))
            bias_t = constp.tile([P, HC], dt.float32)
            nc.scalar.dma_start(bias_t[:], bias_rep[:])
            npad_t = constp.tile([P, NB], dt.bfloat16)
            nc.scalar.dma_start(npad_t[:], npad[:])
            ident = constp.tile([P, P], dt.bfloat16)
            make_identity(nc, ident[:])

            obuf = residp.tile([P, NB * HC], dt.float32)
            obuf16 = residp.tile([P, NB * HC], dt.float16)
            smbuf = residp.tile([P, NB], dt.float32)
            lnb_t = residp.tile([P, NB], dt.float32)

            with (
                tc.tile_pool(name="p2x", bufs=BX) as p2x,
                tc.tile_pool(name="p2", bufs=B2) as p2,
                tc.tile_pool(name="p2w", bufs=BW) as p2w,
                tc.tile_pool(name="p2f", bufs=2) as p2f,
                tc.tile_pool(name="asrcps", bufs=2, space="PSUM") as asrcp,
                tc.tile_pool(name="heps", bufs=BH, space="PSUM") as hepsp,
                tc.tile_pool(name="aggps", bufs=2, space="PSUM") as aggpsp,
            ):
                # DMA prefetch stream: block jb's edge tile is fetched
                # KPRE iterations before its front-end touches it
                xtas = {}
                xoff = 0

                def fetch(jb):
                    nonlocal xoff
                    L = int(L_sched[jb])
                    pool = p2xpre if jb < KPRE else p2x
                    t = pool.tile([P, 2 * L * P], dt.float8e4, tag="xta", name="xta")
                    nc.sync.dma_start(
                        t[:], xeT[xoff:xoff + P * 2 * L * P].rearrange("(a b) -> a b", b=2 * L * P))
                    xoff += P * 2 * L * P
                    xtas[jb] = t

                for jb in range(KPRE):
                    fetch(jb)

                mulctr = 0          # round-robin split of the alpha-multiply
                pend = pend2 = pend3 = None   # deferred h-aggregation chain

                aggbank = {}        # finalize-group index -> PSUM bank tile

                def issue_adst(jbp, xvp):
                    """a_dst into the group bank; runs in its own block (no
                    multiply dependency) so it can never stall, and it is the
                    last xta reader so edge tiles free early. Seven blocks
                    share one 2KB PSUM bank ([P, 7*72] fp32) that the finalize
                    later reads IN PLACE - no PSUM->SBUF park."""
                    g = jbp // GB
                    t0 = jbp - g * GB
                    first = t0 == 0
                    if first:
                        aggbank[g] = aggpsp.tile([P, GB * TDA], dt.float32,
                                                 space="PSUM", tag="agg", name="agg")
                    agg = aggbank[g][:, t0 * TDA:(t0 + 1) * TDA]
                    # start=True on the group's first matmul zeroes the bank
                    nc.tensor.matmul(agg[:, HC + H:TDA], lhsT=xvp[:, :, 0, :], rhs=wtd8_t[:],
                                     perf_mode=DR, start=first, stop=False, skip_group_check=True)

                def issue_hagg(state):
                    """Weighted-h aggregation, deferred three blocks so the
                    alpha-multiplies are long done wherever the list scheduler
                    places these on PE - they must never freeze PE's counting
                    semaphore."""
                    jbp, Lp, wlist, ebp = state
                    g = jbp // GB
                    t0 = jbp - g * GB
                    agg = aggbank[g][:, t0 * TDA:(t0 + 1) * TDA]
                    # softmax denominator: agg[:, 64:72] += I.T @ e_l (PE, not DVE)
                    for l in range(Lp):
                        nc.tensor.matmul(agg[:, HC:HC + H], lhsT=ident[:], rhs=ebp[:, l, :],
                                         start=False, stop=(l == Lp - 1), skip_group_check=True)
                    for ch0, gl, w in wlist:
                        ng2 = gl // 2
                        for i in range(ng2):
                            l = ch0 + 2 * i
                            nc.tensor.matmul(agg[:, 0:HC], lhsT=idr_t[:],
                                             rhs=w[:, 2 * i:2 * i + 2, :], perf_mode=DR,
                                             start=False, stop=(l + 2 >= Lp), skip_group_check=True)
                        if gl % 2:
                            nc.tensor.matmul(agg[:, 0:HC], lhsT=ident[:], rhs=w[:, gl - 1, :],
                                             start=False, stop=(ch0 + gl >= Lp), skip_group_check=True)

                def fin_acts(g0, jb_f, st):
                    """stage 0 [ACT]: exp(leaky(a_dst)) for the pad fix."""
                    av = aggbank[g0 // GB][:].rearrange("p (t d) -> p t d", d=TDA)
                    ng = jb_f + 1 - g0
                    st["lrg"] = lrg = p2f.tile([P, GB, H], dt.float32, tag="lrg", name="lrg")
                    nc.scalar.activation(lrg[:, 0:ng], av[:, 0:ng, HC + H:TDA], AF.Prelu, alpha=NEG_SLOPE)
                    st["edg"] = edg = p2f.tile([P, GB, H], dt.float32, tag="edg", name="edg")
                    nc.scalar.activation(edg[:, 0:ng], lrg[:, 0:ng], AF.Exp, scale=ISC)

                def fin_sden(g0, jb_f, st):
                    """stage 1 [Pool]: sden = s - npad*exp(leaky(a_dst)); no eps
                    needed since s >= e_self > 0."""
                    ng = jb_f + 1 - g0
                    st["pcor"] = pcor = p2f.tile([P, GB, H], dt.float32, tag="pcor", name="pcor")
                    nc.gpsimd.tensor_tensor(
                        out=pcor[:, 0:ng], in0=st["edg"][:, 0:ng],
                        in1=npad_t[:, g0:jb_f + 1].unsqueeze(2).to_broadcast([P, ng, H]),
                        op=OP.mult)
                    av = aggbank[g0 // GB][:].rearrange("p (t d) -> p t d", d=TDA)
                    st["sden"] = sden = p2f.tile([P, GB, H], dt.float32, tag="sden", name="sden")
                    nc.vector.tensor_tensor(out=sden[:, 0:ng], in0=av[:, 0:ng, HC:HC + H],
                                            in1=pcor[:, 0:ng], op=OP.subtract)

                def fin_srec(g0, jb_f, st):
                    """stage 2 [DVE]: reciprocal of the softmax denominator."""
                    ng = jb_f + 1 - g0
                    st["srec"] = srec = p2f.tile([P, GB, H], dt.float32, tag="srec", name="srec")
                    nc.vector.reciprocal(srec[:, 0:ng], st["sden"][:, 0:ng])

                def fin_ov(g0, jb_f, st):
                    """stage 3 [Pool]: onorm = h * (srec/4); /4 undoes the x4
                    h-weight scale."""
                    av = aggbank[g0 // GB][:].rearrange("p (t d) -> p t d", d=TDA)
                    ng = jb_f + 1 - g0
                    ov = obuf[:, g0 * HC:(jb_f + 1) * HC].rearrange("p (t d) -> p t d", d=HC)
                    nc.vector.scalar_tensor_tensor(
                        out=ov.rearrange("p t (h c) -> p t h c", c=C),
                        in0=st["srec"][:, 0:ng].unsqueeze(3).to_broadcast([P, ng, H, C]),
                        scalar=1.0 / SC_H,
                        in1=av[:, 0:ng, 0:HC].rearrange("p t (h c) -> p t h c", c=C),
                        op0=OP.mult, op1=OP.mult)
                    if has_bias:
                        nc.gpsimd.tensor_tensor(
                            out=ov, in0=ov,
                            in1=bias_t[:].unsqueeze(1).to_broadcast([P, ng, HC]), op=OP.add)

                def fin_exf(g0, jb_f, st):
                    """stage 4 [ACT]: exp for the log-softmax row sums."""
                    ng = jb_f + 1 - g0
                    ov = obuf[:, g0 * HC:(jb_f + 1) * HC].rearrange("p (t d) -> p t d", d=HC)
                    st["exf"] = exf = p2f.tile([P, GB, HC], dt.bfloat16, tag="exf", name="exf")
                    nc.scalar.activation(exf[:, 0:ng], ov, AF.Exp)

                def fin_sum(g0, jb_f, st):
                    """stage 5 [DVE]: log-softmax row sums."""
                    ng = jb_f + 1 - g0
                    nc.vector.tensor_reduce(
                        smbuf[:, g0:jb_f + 1].unsqueeze(2), st["exf"][:, 0:ng],
                        axis=mybir.AxisListType.X, op=OP.add)

                def fl_ln(b0, b1):
                    nc.scalar.activation(lnb_t[:, b0:b1], smbuf[:, b0:b1], AF.Ln)

                def fl_sub(b0, b1, eng=None):
                    with nc.allow_low_precision(reason="fp16 output"):
                        (eng or nc.vector).tensor_tensor(
                            out=obuf16[:, b0 * HC:b1 * HC].rearrange("p (t d) -> p t d", d=HC),
                            in0=obuf[:, b0 * HC:b1 * HC].rearrange("p (t d) -> p t d", d=HC),
                            in1=lnb_t[:, b0:b1].unsqueeze(2).to_broadcast([P, b1 - b0, HC]),
                            op=OP.subtract)

                def fl_dma(b0, b1):
                    nc.sync.dma_start(out[:, b0 * HC:b1 * HC], obuf16[:, b0 * HC:b1 * HC])

                # software-pipelined finalize/flush: each cross-engine stage is
                # issued one block after its producer so no in-order engine
                # queue ever head-of-line blocks on a not-yet-ready input.
                sched = {}

                def at(jbx, fn, *args):
                    sched.setdefault(jbx, []).append((fn, args))

                FIN = [fin_acts, fin_sden, fin_srec, fin_ov, fin_exf, fin_sum]
                groups = [(jb_f + 1 - GB, jb_f) for jb_f in range(GB - 1, NB, GB)]
                if groups[-1][1] != NB - 1:
                    groups.append((groups[-1][1] + 1, NB - 1))
                for g0, jb_f in groups:
                    st = {}
                    for k, fnk in enumerate(FIN):
                        at(jb_f + 4 + k, fnk, g0, jb_f, st)
                # Log-softmax flushes batch near the END: Ln lives in a
                # different ACT table set than Prelu/Exp, so in-loop flushes
                # would thrash 1.3us table reloads (and freeze ACT's queue)
                # twice per flush. One switch for blocks [0,84) overlaps the
                # last compute; the rest goes after the final row sums.
                at(NB - 2, fl_ln, 0, NB - FB)
                for i, b1 in enumerate(range(FB, NB - FB + 1, FB)):
                    at(NB - 1, fl_sub, b1 - FB, b1, nc.gpsimd if i % 2 == 0 else nc.vector)
                    at(NB - 1, fl_dma, b1 - FB, b1)
                at(NB + 10, fl_ln, NB - FB, NB)
                at(NB + 11, fl_sub, NB - FB, NB)
                at(NB + 11, fl_dma, NB - FB, NB)

                def frontend(jb):
                    """Block front-end: a_src PSUM bank + e = exp(leaky(z)).
                    Issued two blocks AHEAD of the h-groups so the alpha
                    weights are long ready when the multiplies dispatch -
                    PSUM bank recycling then runs at pure engine throughput."""
                    L = int(L_sched[jb])
                    xv = xtas[jb][:].rearrange("p (k l q) -> p k l q", k=2, q=P)

                    # z = a_src + a_dst directly in one PSUM bank: the l=0
                    # (self-loop) slice carries x_own, so it pre-fills a_dst
                    # via a broadcast-tiled Wtd rhs; DoubleRow a_src follows
                    asrc = asrcp.tile([P, L * H], dt.float32, space="PSUM", tag="asrc", name="asrc")
                    nc.tensor.matmul(asrc[:], lhsT=xv[:, 0, 0, :],
                                     rhs=wdb_t[:, 0, :].unsqueeze(1).to_broadcast([P, L, H]),
                                     start=True, stop=False, skip_group_check=True)
                    nc.tensor.matmul(asrc[:], lhsT=xv[:, 1, 0, :],
                                     rhs=wdb_t[:, 1, :].unsqueeze(1).to_broadcast([P, L, H]),
                                     start=False, stop=False, skip_group_check=True)
                    for l in range(L):
                        nc.tensor.matmul(asrc[:, l * H:(l + 1) * H],
                                         lhsT=xv[:, :, l, :], rhs=was8_t[:],
                                         perf_mode=DR,
                                         start=False, stop=(l == L - 1), skip_group_check=True)
                    # e = exp(leaky(z)/4) straight from PSUM, stored head-major
                    # [P, H, L] so the softmax row-sum reduces the inner axis
                    lr = p2.tile([P, L, H], dt.float32, tag="lr", name="lr")
                    nc.scalar.activation(lr[:], asrc[:].rearrange("p (l h) -> p l h", h=H),
                                         AF.Prelu, alpha=NEG_SLOPE)
                    ebt = p2.tile([P, L, H], dt.bfloat16, tag="ebt", name="ebt")
                    nc.scalar.activation(ebt[:], lr[:], AF.Exp, scale=ISC)
                    return jb, xv, ebt

                fes = {0: frontend(0)}
                fes[1] = frontend(1)
                fes[2] = frontend(2)
                for jb in range(NB):
                    L = int(L_sched[jb])
                    _, xv, ebt = fes.pop(jb)

                    if jb + KPRE < NB:
                        fetch(jb + KPRE)
                    # front-end TWO blocks ahead: by the time block jb+2's
                    # multiplies are needed, its alpha weights are ~2 block
                    # periods old. high_priority makes the list scheduler emit
                    # these BEFORE older blocks' aggregation matmuls on PE
                    if jb + 3 < NB:
                        with tc.high_priority(offset=200):
                            fes[jb + 3] = frontend(jb + 3)

                    # h projection per 16-edge superblock (two PSUM banks,
                    # one accumulation group per bank) + ONE alpha-weighting
                    # multiply per superblock to halve DVE call overhead
                    wlist = []
                    for ch0 in range(0, L, 2 * GP8):
                        gl = min(2 * GP8, L - ch0)
                        ps = hepsp.tile([P, gl * HC], dt.float32, space="PSUM", tag="heps", name="heps")
                        for li in range(gl):
                            l = ch0 + li
                            nc.tensor.matmul(ps[:, li * HC:(li + 1) * HC],
                                             lhsT=xv[:, :, l, :], rhs=wf8_t[:],
                                             perf_mode=DR,
                                             start=(li % GP8 == 0),
                                             stop=(li == gl - 1 or li == GP8 - 1),
                                             skip_group_check=True)
                        w = p2w.tile([P, 2 * GP8, HC], dt.float8e4, tag="w", name="w")
                        mulctr += 1
                        nc.vector.tensor_tensor(
                            out=w[:, 0:gl].rearrange("p l (h c) -> p l h c", c=C),
                            in0=ps[:].rearrange("p (l h c) -> p l h c", h=H, c=C),
                            in1=ebt[:, ch0:ch0 + gl, :]
                                .unsqueeze(3).to_broadcast([P, gl, H, C]),
                            op=OP.mult)
                        wlist.append((ch0, gl, w))

                    # a_dst for this block (no multiply dependency)
                    issue_adst(jb, xv)

                    # weighted-h + e aggregation for block jb-3
                    if pend3 is not None:
                        issue_hagg(pend3)
                    pend3 = pend2
                    pend2 = pend
                    pend = (jb, L, wlist, ebt)

                    # pipelined finalize/flush stages due at this block
                    for fnk, args in sched.pop(jb, []):
                        fnk(*args)

                # tail: last block's aggregation, then drain the remaining
                # pipelined finalize/flush stages in schedule order
                for st_ in (pend3, pend2, pend):
                    if st_ is not None:
                        issue_hagg(st_)
                for jbx in sorted(sched):
                    for fnk, args in sched[jbx]:
                        fnk(*args)

    nc.compile()
    return nc


def kernel(x, edge_index, W, att_src, att_dst, bias):
    in_maps, L_sched, S, row2node, has_bias = _host_prep(x, edge_index, W, att_src, att_dst, bias)
    nc = _build_program(L_sched, S, has_bias=has_bias)
    res = run_bass_kernel_spmd(nc, in_maps, core_ids=list(range(NCORES)))
    out_full = np.empty((N, HC), dtype=np.float32)
    for cc in range(NCORES):
        o = np.asarray(res.results[cc]["out"]).astype(np.float32)   # [128, NB*HC]
        o = o.reshape(P, NB, HC).transpose(1, 0, 2).reshape(NPC, HC)
        rr = row2node[cc * NPC:(cc + 1) * NPC]
        m = rr >= 0
        out_full[rr[m]] = o[m]
    return out_full
